# revision 4
# baseline (speedup 1.0000x reference)
"""SVRaster (sparse-voxel NeRF-style raymarcher) for 8x Trainium2 NeuronCores.

Data-parallel over rays: each of the 8 cores renders NUM_RAYS/8 rays against
the full (replicated) voxel tables.  Per (ray, sample) the kernel computes the
voxel index on-chip, gathers density+SH coefficients with one indirect DMA row
per sample point, and does SH shading + alpha compositing with DVE/ACT ops
(compositing cumprod via tensor_tensor_scan, final weighted reduction via
scalar_tensor_tensor accum_out).

Self-contained: hardcodes shapes from the problem spec.
"""

import numpy as np

P = 128                     # SBUF partitions == rays per batch
S = 128                     # samples per ray
RES = 128                   # voxel grid resolution
V = RES ** 3
NEAR, FAR = 0.05, 4.0
NUM_RAYS = 32768
N_CORES = 8
TABW = 28                   # 1 density + 27 SH coeffs per voxel row

# Spherical-harmonic basis constants (degree 2)
SH0 = 0.28209479177387814
SH1 = 0.4886025119029199
SH2 = 1.0925484305920792
SH3 = 0.31539156525252005
SH4 = 0.5462742152960396

_CACHE = {}


def build(n_rays):
    """Build the Bass/Tile program for one core processing n_rays rays."""
    import concourse.bass as bass
    import concourse.bacc as bacc
    import concourse.mybir as mybir
    import concourse.tile as tile

    f32 = mybir.dt.float32
    i32 = mybir.dt.int32
    Alu = mybir.AluOpType
    Act = mybir.ActivationFunctionType

    assert n_rays % P == 0
    B = n_rays // P
    DT = float(np.float32((FAR - NEAR) / (S - 1)))

    nc = bacc.Bacc("TRN2", target_bir_lowering=False)
    rays_h = nc.dram_tensor("rays", [n_rays, 6], f32, kind="ExternalInput")
    tab_h = nc.dram_tensor("tab", [V, TABW], f32, kind="ExternalInput")
    tvec_h = nc.dram_tensor("tvec", [1, S], f32, kind="ExternalInput")
    out_h = nc.dram_tensor("out", [n_rays, 3], f32, kind="ExternalOutput")

    with tile.TileContext(nc) as tc:
        with (
            tc.tile_pool(name="const", bufs=1) as cpool,
            tc.tile_pool(name="work", bufs=3) as wpool,
            tc.tile_pool(name="gath", bufs=4) as gpool,
        ):
            # ---- one-time setup ----
            rays_t = cpool.tile([P, B, 6], f32)
            nc.sync.dma_start(
                out=rays_t[:], in_=rays_h[:].rearrange("(b p) c -> p b c", p=P)
            )
            tb_t = cpool.tile([P, S], f32)  # t value broadcast to all partitions
            nc.sync.dma_start(out=tb_t[:], in_=tvec_h[:].to_broadcast([P, S]))

            # basis [P, 9, B]
            bas_t = cpool.tile([P, 9, B], f32)
            t1 = cpool.tile([P, B], f32)
            xs = rays_t[:, :, 3]
            ys = rays_t[:, :, 4]
            zs = rays_t[:, :, 5]
            nc.vector.memset(bas_t[:, 0, :], SH0)
            nc.vector.tensor_scalar(bas_t[:, 1, :], ys, -SH1, None, Alu.mult)
            nc.vector.tensor_scalar(bas_t[:, 2, :], zs, SH1, None, Alu.mult)
            nc.vector.tensor_scalar(bas_t[:, 3, :], xs, -SH1, None, Alu.mult)
            nc.vector.scalar_tensor_tensor(bas_t[:, 4, :], xs, SH2, ys, Alu.mult, Alu.mult)
            nc.vector.scalar_tensor_tensor(bas_t[:, 5, :], ys, -SH2, zs, Alu.mult, Alu.mult)
            nc.vector.scalar_tensor_tensor(t1[:], zs, 3.0 * SH3, zs, Alu.mult, Alu.mult)
            nc.vector.tensor_scalar(bas_t[:, 6, :], t1[:], -SH3, None, Alu.add)
            nc.vector.scalar_tensor_tensor(bas_t[:, 7, :], xs, -SH2, zs, Alu.mult, Alu.mult)
            t2 = cpool.tile([P, B], f32)
            nc.vector.scalar_tensor_tensor(t2[:], xs, SH4, xs, Alu.mult, Alu.mult)
            nc.vector.scalar_tensor_tensor(bas_t[:, 8, :], ys, -SH4, ys, Alu.mult, Alu.mult)
            nc.vector.tensor_tensor(bas_t[:, 8, :], bas_t[:, 8, :], t2[:], Alu.add)

            res_t = cpool.tile([P, B, 3], f32)

            # ---- per ray-batch ----
            for b in range(B):
                # u = ((o + t*d) + 1) * 64, computed with reference-matching
                # rounding order, per coordinate
                u3 = wpool.tile([P, 3, S], f32, tag="u3")
                for c in range(3):
                    o_ap = rays_t[:, b, c:c + 1]
                    d_ap = rays_t[:, b, 3 + c:4 + c]
                    nc.vector.tensor_scalar(
                        u3[:, c, :], tb_t[:], d_ap, o_ap, Alu.mult, Alu.add
                    )
                    nc.vector.tensor_scalar(
                        u3[:, c, :], u3[:, c, :], 1.0, 64.0, Alu.add, Alu.mult
                    )
                # floor(u) via convert + round-trip correction (works for both
                # truncating sim and round-to-nearest HW converters):
                # ff = cvt_back(cvt(u)) - (cvt_back > u)
                iv3 = wpool.tile([P, 3, S], i32, tag="iv3")
                nc.vector.tensor_copy(iv3[:], u3[:])
                fb3 = wpool.tile([P, 3, S], f32, tag="fb3")
                nc.vector.tensor_copy(fb3[:], iv3[:])
                gt3 = wpool.tile([P, 3, S], f32, tag="gt3")
                nc.vector.tensor_tensor(gt3[:], fb3[:], u3[:], Alu.is_gt)
                ff3 = wpool.tile([P, 3, S], f32, tag="ff3")
                nc.vector.tensor_tensor(ff3[:], fb3[:], gt3[:], Alu.subtract)
                # clip (exact integers in f32)
                cf3 = wpool.tile([P, 3, S], f32, tag="cf3")
                nc.vector.tensor_scalar(cf3[:], ff3[:], 0.0, 127.0, Alu.max, Alu.min)
                # in-bounds: floor == clipped floor, per coord, ANDed
                meq = wpool.tile([P, 3, S], f32, tag="meq")
                nc.vector.tensor_tensor(meq[:], ff3[:], cf3[:], Alu.is_equal)
                mask = wpool.tile([P, S], f32, tag="mask")
                nc.vector.tensor_tensor(mask[:], meq[:, 0, :], meq[:, 1, :], Alu.mult)
                nc.vector.tensor_tensor(mask[:], mask[:], meq[:, 2, :], Alu.mult)
                # flat voxel index (exact in f32, <2^24), then to int32
                flatf = wpool.tile([P, S], f32, tag="flatf")
                nc.vector.scalar_tensor_tensor(
                    flatf[:], cf3[:, 0, :], 128.0, cf3[:, 1, :], Alu.mult, Alu.add
                )
                nc.vector.scalar_tensor_tensor(
                    flatf[:], flatf[:], 128.0, cf3[:, 2, :], Alu.mult, Alu.add
                )
                idx = wpool.tile([P, S], i32, tag="idx")
                nc.vector.tensor_copy(idx[:], flatf[:])

                # gather [P, S, 28] rows from the combined table; the HW
                # indirect-DMA lowering supports one offset per partition per
                # call, so issue one call per sample column
                gath = gpool.tile([P, S, TABW], f32, tag="gath")
                for s in range(S):
                    nc.gpsimd.indirect_dma_start(
                        out=gath[:, s, :],
                        out_offset=None,
                        in_=tab_h[:],
                        in_offset=bass.IndirectOffsetOnAxis(ap=idx[:, s:s + 1], axis=0),
                    )

                # density -> transmittance factors q = exp(-dt*mask*exp(den))
                e_t = wpool.tile([P, S], f32, tag="e_t")
                nc.scalar.activation(e_t[:], gath[:, :, 0], Act.Exp)
                nege = wpool.tile([P, S], f32, tag="nege")
                nc.vector.scalar_tensor_tensor(
                    nege[:], e_t[:], -DT, mask[:], Alu.mult, Alu.mult
                )
                q_t = wpool.tile([P, S], f32, tag="q_t")
                nc.scalar.activation(q_t[:], nege[:], Act.Exp)
                # inclusive cumprod via scan; T[:,0]=1 so T[:,s] = prod_{s'<s} q
                T_t = wpool.tile([P, S + 1], f32, tag="T_t")
                nc.vector.memset(T_t[:, 0:1], 1.0)
                nc.vector.tensor_tensor_scan(
                    T_t[:, 1:S + 1], q_t[:], q_t[:], 1.0, Alu.mult, Alu.bypass
                )
                w_t = wpool.tile([P, S], f32, tag="w_t")
                nc.vector.tensor_tensor(
                    w_t[:], T_t[:, 0:S], T_t[:, 1:S + 1], Alu.subtract
                )

                # SH shading: acc[r,s,c] = sum_k basis[r,k]*sh[r,s,k,c]
                acc = wpool.tile([P, S, 3], f32, tag="acc")
                nc.vector.tensor_scalar(acc[:], gath[:, :, 1:4], SH0, None, Alu.mult)
                for k in range(1, 9):
                    nc.vector.scalar_tensor_tensor(
                        acc[:],
                        gath[:, :, 1 + 3 * k:4 + 3 * k],
                        bas_t[:, k, b:b + 1],
                        acc[:],
                        Alu.mult,
                        Alu.add,
                    )
                # sigmoid via exp + reciprocal so ACT only ever needs the Exp
                # table (avoids per-batch activation-table reloads)
                rgbs = wpool.tile([P, S, 3], f32, tag="rgbs")
                nc.scalar.activation(rgbs[:], acc[:], Act.Exp, scale=-1.0)
                nc.vector.tensor_scalar(rgbs[:], rgbs[:], 1.0, None, Alu.add)
                nc.vector.reciprocal(rgbs[:], rgbs[:])

                # weighted reduction over samples
                scr = wpool.tile([P, 3, S], f32, tag="scr")
                for c in range(3):
                    nc.vector.scalar_tensor_tensor(
                        scr[:, c, :],
                        rgbs[:, :, c],
                        1.0,
                        w_t[:],
                        Alu.mult,
                        Alu.mult,
                        accum_out=res_t[:, b, c:c + 1],
                    )

            nc.sync.dma_start(
                out=out_h[:].rearrange("(b p) c -> p b c", p=P), in_=res_t[:]
            )

    nc.compile()
    return nc


def _host_prep(rays_o, rays_d, voxel_density, voxel_sh):
    import jax.numpy as jnp

    tab = np.empty((V, TABW), dtype=np.float32)
    tab[:, 0] = np.asarray(voxel_density, dtype=np.float32)
    tab[:, 1:] = np.asarray(voxel_sh, dtype=np.float32)
    rays = np.concatenate(
        [np.asarray(rays_o, np.float32), np.asarray(rays_d, np.float32)], axis=1
    )  # [NUM_RAYS, 6]
    tvec = np.asarray(
        jnp.linspace(NEAR, FAR, S, dtype=jnp.float32), dtype=np.float32
    ).reshape(1, S)
    return rays, tab, tvec


def kernel(rays_o, rays_d, voxel_density, voxel_sh):
    from concourse.bass_utils import run_bass_kernel_spmd

    n_rays = rays_o.shape[0]
    per_core = n_rays // N_CORES
    rays, tab, tvec = _host_prep(rays_o, rays_d, voxel_density, voxel_sh)

    key = ("nc", per_core)
    if key not in _CACHE:
        _CACHE[key] = build(per_core)
    nc = _CACHE[key]

    in_maps = [
        {
            "rays": np.ascontiguousarray(rays[i * per_core:(i + 1) * per_core]),
            "tab": tab,
            "tvec": tvec,
        }
        for i in range(N_CORES)
    ]
    res = run_bass_kernel_spmd(nc, in_maps, core_ids=list(range(N_CORES)))
    out = np.concatenate([r["out"] for r in res.results], axis=0)
    return out


# revision 5
# speedup vs baseline: 1.6183x; 1.6183x over previous
"""SVRaster (sparse-voxel NeRF-style raymarcher) for 8x Trainium2 NeuronCores.

Data-parallel over rays: each of the 8 cores renders NUM_RAYS/8 rays against
the full (replicated) voxel tables.  Per ray, only a per-ray window of S_w
samples (covering the ray/scene-box intersection) is processed: the voxel
index is computed on-chip, density+SH coefficients are fetched with one
128-row indirect DMA per sample column, and SH shading + alpha compositing
run on DVE/ACT (compositing cumprod via tensor_tensor_scan, final weighted
reduction via scalar_tensor_tensor accum_out).

Self-contained: hardcodes shapes from the problem spec.
"""

import numpy as np

P = 128                     # SBUF partitions == rays per batch
S = 128                     # samples per ray
RES = 128                   # voxel grid resolution
V = RES ** 3
NEAR, FAR = 0.05, 4.0
NUM_RAYS = 32768
N_CORES = 8
TABW = 28                   # 1 density + 27 SH coeffs per voxel row

# Spherical-harmonic basis constants (degree 2)
SH0 = 0.28209479177387814
SH1 = 0.4886025119029199
SH2 = 1.0925484305920792
SH3 = 0.31539156525252005
SH4 = 0.5462742152960396

_CACHE = {}


def build(n_rays, s_w):
    """Bass/Tile program for one core: n_rays rays, s_w-sample windows."""
    import concourse.bass as bass
    import concourse.bacc as bacc
    import concourse.mybir as mybir
    import concourse.tile as tile

    f32 = mybir.dt.float32
    i32 = mybir.dt.int32
    Alu = mybir.AluOpType
    Act = mybir.ActivationFunctionType

    assert n_rays % P == 0
    B = n_rays // P
    DT = float(np.float32((FAR - NEAR) / (S - 1)))
    SW = s_w

    nc = bacc.Bacc("TRN2", target_bir_lowering=False)
    rays_h = nc.dram_tensor("rays", [n_rays, 6], f32, kind="ExternalInput")
    tab_h = nc.dram_tensor("tab", [V, TABW], f32, kind="ExternalInput")
    tvec_h = nc.dram_tensor("tvec", [S + SW, 1], f32, kind="ExternalInput")
    out_h = nc.dram_tensor("out", [n_rays, 3], f32, kind="ExternalOutput")

    def floorchain(pool, src, tagp):
        """floor(src) in f32, correct for truncating or rounding converters."""
        iv = pool.tile(list(src.shape), i32, tag=tagp + "_iv")
        nc.vector.tensor_copy(iv[:], src)
        fb = pool.tile(list(src.shape), f32, tag=tagp + "_fb")
        nc.vector.tensor_copy(fb[:], iv[:])
        gt = pool.tile(list(src.shape), f32, tag=tagp + "_gt")
        nc.vector.tensor_tensor(gt[:], fb[:], src, Alu.is_gt)
        nc.vector.tensor_tensor(fb[:], fb[:], gt[:], Alu.subtract)
        return fb

    with tile.TileContext(nc) as tc:
        with (
            tc.tile_pool(name="const", bufs=1) as cpool,
            tc.tile_pool(name="work", bufs=3) as wpool,
            tc.tile_pool(name="gath", bufs=4) as gpool,
        ):
            # ---- one-time setup ----
            rays_t = cpool.tile([P, B, 6], f32)
            nc.sync.dma_start(
                out=rays_t[:], in_=rays_h[:].rearrange("(b p) c -> p b c", p=P)
            )
            # basis [P, 9, B]
            bas_t = cpool.tile([P, 9, B], f32)
            t1 = cpool.tile([P, B], f32)
            xs = rays_t[:, :, 3]
            ys = rays_t[:, :, 4]
            zs = rays_t[:, :, 5]
            nc.vector.memset(bas_t[:, 0, :], SH0)
            nc.vector.tensor_scalar(bas_t[:, 1, :], ys, -SH1, None, Alu.mult)
            nc.vector.tensor_scalar(bas_t[:, 2, :], zs, SH1, None, Alu.mult)
            nc.vector.tensor_scalar(bas_t[:, 3, :], xs, -SH1, None, Alu.mult)
            nc.vector.scalar_tensor_tensor(bas_t[:, 4, :], xs, SH2, ys, Alu.mult, Alu.mult)
            nc.vector.scalar_tensor_tensor(bas_t[:, 5, :], ys, -SH2, zs, Alu.mult, Alu.mult)
            nc.vector.scalar_tensor_tensor(t1[:], zs, 3.0 * SH3, zs, Alu.mult, Alu.mult)
            nc.vector.tensor_scalar(bas_t[:, 6, :], t1[:], -SH3, None, Alu.add)
            nc.vector.scalar_tensor_tensor(bas_t[:, 7, :], xs, -SH2, zs, Alu.mult, Alu.mult)
            t2 = cpool.tile([P, B], f32)
            nc.vector.scalar_tensor_tensor(t2[:], xs, SH4, xs, Alu.mult, Alu.mult)
            nc.vector.scalar_tensor_tensor(bas_t[:, 8, :], ys, -SH4, ys, Alu.mult, Alu.mult)
            nc.vector.tensor_tensor(bas_t[:, 8, :], bas_t[:, 8, :], t2[:], Alu.add)

            # window math coefficients: u_c(s) = A_c*s + B_c (f32 approx,
            # used only for conservative window placement)
            A3 = cpool.tile([P, B, 3], f32)
            B3 = cpool.tile([P, B, 3], f32)
            o_view = rays_t[:, :, 0:3]
            d_view = rays_t[:, :, 3:6]
            nc.vector.tensor_scalar(A3[:], d_view, 64.0 * DT, None, Alu.mult)
            tmp3 = cpool.tile([P, B, 3], f32)
            nc.vector.tensor_scalar(tmp3[:], o_view, 64.0, 64.0, Alu.mult, Alu.add)
            nc.vector.scalar_tensor_tensor(
                B3[:], d_view, 64.0 * NEAR, tmp3[:], Alu.mult, Alu.add
            )

            res_t = cpool.tile([P, B, 3], f32)

            # ---- per ray-batch ----
            for b in range(B):
                # conservative per-ray window start a = clip(floor(enter)-2)
                negB = wpool.tile([P, 3], f32, tag="negB")
                nc.vector.tensor_scalar(negB[:], B3[:, b, :], -1.0, None, Alu.mult)
                rA = wpool.tile([P, 3], f32, tag="rA")
                nc.vector.reciprocal(rA[:], A3[:, b, :])
                sA = wpool.tile([P, 3], f32, tag="sA")
                nc.vector.tensor_tensor(sA[:], negB[:], rA[:], Alu.mult)
                sB = wpool.tile([P, 3], f32, tag="sB")
                nc.vector.tensor_scalar(sB[:], negB[:], 128.0, None, Alu.add)
                nc.vector.tensor_tensor(sB[:], sB[:], rA[:], Alu.mult)
                en = wpool.tile([P, 3], f32, tag="en")
                nc.vector.tensor_tensor(en[:], sA[:], sB[:], Alu.min)
                enm = wpool.tile([P, 1], f32, tag="enm")
                nc.vector.tensor_reduce(enm[:], en[:], mybir.AxisListType.X, Alu.max)
                nc.vector.tensor_scalar(enm[:], enm[:], -2.0, 0.0, Alu.add, Alu.max)
                nc.vector.tensor_scalar(enm[:], enm[:], float(S - 1), None, Alu.min)
                a_i = wpool.tile([P, 1], i32, tag="a_i")
                nc.vector.tensor_copy(a_i[:], enm[:])  # any-rounding: ±1 ok

                # exact t values for this ray's window: t_til[p,j]=tvec[a_p+j]
                t_til = wpool.tile([P, SW], f32, tag="t_til")
                nc.gpsimd.indirect_dma_start(
                    out=t_til[:],
                    out_offset=None,
                    in_=tvec_h[:],
                    in_offset=bass.IndirectOffsetOnAxis(ap=a_i[:], axis=0),
                )

                # u = ((o + t*d) + 1) * 64 with reference rounding order
                u3 = wpool.tile([P, 3, SW], f32, tag="u3")
                for c in range(3):
                    o_ap = rays_t[:, b, c:c + 1]
                    d_ap = rays_t[:, b, 3 + c:4 + c]
                    nc.vector.tensor_scalar(
                        u3[:, c, :], t_til[:], d_ap, o_ap, Alu.mult, Alu.add
                    )
                    nc.vector.tensor_scalar(
                        u3[:, c, :], u3[:, c, :], 1.0, 64.0, Alu.add, Alu.mult
                    )
                ff3 = floorchain(wpool, u3[:], "f3")
                cf3 = wpool.tile([P, 3, SW], f32, tag="cf3")
                nc.vector.tensor_scalar(cf3[:], ff3[:], 0.0, 127.0, Alu.max, Alu.min)
                meq = wpool.tile([P, 3, SW], f32, tag="meq")
                nc.vector.tensor_tensor(meq[:], ff3[:], cf3[:], Alu.is_equal)
                mask = wpool.tile([P, SW], f32, tag="mask")
                nc.vector.tensor_tensor(mask[:], meq[:, 0, :], meq[:, 1, :], Alu.mult)
                nc.vector.tensor_tensor(mask[:], mask[:], meq[:, 2, :], Alu.mult)
                flatf = wpool.tile([P, SW], f32, tag="flatf")
                nc.vector.scalar_tensor_tensor(
                    flatf[:], cf3[:, 0, :], 128.0, cf3[:, 1, :], Alu.mult, Alu.add
                )
                nc.vector.scalar_tensor_tensor(
                    flatf[:], flatf[:], 128.0, cf3[:, 2, :], Alu.mult, Alu.add
                )
                idx = wpool.tile([P, SW], i32, tag="idx")
                nc.vector.tensor_copy(idx[:], flatf[:])

                # gather [P, SW, 28] voxel rows (one 128-row call per column)
                gath = gpool.tile([P, SW, TABW], f32, tag="gath")
                for s in range(SW):
                    nc.gpsimd.indirect_dma_start(
                        out=gath[:, s, :],
                        out_offset=None,
                        in_=tab_h[:],
                        in_offset=bass.IndirectOffsetOnAxis(ap=idx[:, s:s + 1], axis=0),
                    )

                # density -> q = exp(-dt*mask*exp(den))
                e_t = wpool.tile([P, SW], f32, tag="e_t")
                nc.scalar.activation(e_t[:], gath[:, :, 0], Act.Exp)
                nege = wpool.tile([P, SW], f32, tag="nege")
                nc.vector.scalar_tensor_tensor(
                    nege[:], e_t[:], -DT, mask[:], Alu.mult, Alu.mult
                )
                q_t = wpool.tile([P, SW], f32, tag="q_t")
                nc.scalar.activation(q_t[:], nege[:], Act.Exp)
                T_t = wpool.tile([P, SW + 1], f32, tag="T_t")
                nc.vector.memset(T_t[:, 0:1], 1.0)
                nc.vector.tensor_tensor_scan(
                    T_t[:, 1:SW + 1], q_t[:], q_t[:], 1.0, Alu.mult, Alu.bypass
                )
                w_t = wpool.tile([P, SW], f32, tag="w_t")
                nc.vector.tensor_tensor(
                    w_t[:], T_t[:, 0:SW], T_t[:, 1:SW + 1], Alu.subtract
                )

                # SH shading
                acc = wpool.tile([P, SW, 3], f32, tag="acc")
                nc.vector.tensor_scalar(acc[:], gath[:, :, 1:4], SH0, None, Alu.mult)
                for k in range(1, 9):
                    nc.vector.scalar_tensor_tensor(
                        acc[:],
                        gath[:, :, 1 + 3 * k:4 + 3 * k],
                        bas_t[:, k, b:b + 1],
                        acc[:],
                        Alu.mult,
                        Alu.add,
                    )
                # sigmoid via exp + reciprocal (single ACT table)
                rgbs = wpool.tile([P, SW, 3], f32, tag="rgbs")
                nc.scalar.activation(rgbs[:], acc[:], Act.Exp, scale=-1.0)
                nc.vector.tensor_scalar(rgbs[:], rgbs[:], 1.0, None, Alu.add)
                nc.vector.reciprocal(rgbs[:], rgbs[:])

                # weighted reduction over window samples
                scr = wpool.tile([P, 3, SW], f32, tag="scr")
                for c in range(3):
                    nc.vector.scalar_tensor_tensor(
                        scr[:, c, :],
                        rgbs[:, :, c],
                        1.0,
                        w_t[:],
                        Alu.mult,
                        Alu.mult,
                        accum_out=res_t[:, b, c:c + 1],
                    )

            nc.sync.dma_start(
                out=out_h[:].rearrange("(b p) c -> p b c", p=P), in_=res_t[:]
            )

    nc.compile()
    return nc


def _window_width(rays_o, rays_d):
    """Max per-ray in-bounds sample span (conservative, from actual rays)."""
    o = np.asarray(rays_o, np.float64)
    d = np.asarray(rays_d, np.float64)
    t = np.linspace(NEAR, FAR, S)
    u = (o[:, None, :] + d[:, None, :] * t[None, :, None] + 1.0) * 64.0
    inb = np.all((u >= 0) & (u < 128), axis=-1)  # [R, S] approx in-bounds
    any_r = inb.any(axis=1)
    first = np.where(any_r, inb.argmax(axis=1), 0)
    last = np.where(any_r, S - 1 - inb[:, ::-1].argmax(axis=1), 0)
    span = int((last - first + 1).max()) if any_r.any() else 1
    sw = min(S, span + 6)
    return max(sw, 8)


def _host_prep(rays_o, rays_d, voxel_density, voxel_sh, s_w):
    import jax.numpy as jnp

    tab = np.empty((V, TABW), dtype=np.float32)
    tab[:, 0] = np.asarray(voxel_density, dtype=np.float32)
    tab[:, 1:] = np.asarray(voxel_sh, dtype=np.float32)
    rays = np.concatenate(
        [np.asarray(rays_o, np.float32), np.asarray(rays_d, np.float32)], axis=1
    )  # [NUM_RAYS, 6]
    tvec = np.full((S + s_w, 1), 1e9, dtype=np.float32)
    tvec[:S, 0] = np.asarray(
        jnp.linspace(NEAR, FAR, S, dtype=jnp.float32), dtype=np.float32
    )
    return rays, tab, tvec


def kernel(rays_o, rays_d, voxel_density, voxel_sh):
    from concourse.bass_utils import run_bass_kernel_spmd

    n_rays = rays_o.shape[0]
    per_core = n_rays // N_CORES
    s_w = _window_width(rays_o, rays_d)
    rays, tab, tvec = _host_prep(rays_o, rays_d, voxel_density, voxel_sh, s_w)

    key = ("nc", per_core, s_w)
    if key not in _CACHE:
        _CACHE[key] = build(per_core, s_w)
    nc = _CACHE[key]

    in_maps = [
        {
            "rays": np.ascontiguousarray(rays[i * per_core:(i + 1) * per_core]),
            "tab": tab,
            "tvec": tvec,
        }
        for i in range(N_CORES)
    ]
    res = run_bass_kernel_spmd(nc, in_maps, core_ids=list(range(N_CORES)))
    out = np.concatenate([r["out"] for r in res.results], axis=0)
    return out


# revision 10
# speedup vs baseline: 1.8111x; 1.1191x over previous
"""SVRaster (sparse-voxel NeRF-style raymarcher) for 8x Trainium2 NeuronCores.

Data-parallel over rays: each of the 8 cores renders NUM_RAYS/8 rays against
the full (replicated) voxel tables.  Per ray, only a per-ray window of S_w
samples (covering the ray/scene-box intersection) is processed: the voxel
index is computed on-chip, density+SH coefficients are fetched with one
128-row indirect DMA per sample column, and SH shading + alpha compositing
run on DVE/ACT (compositing cumprod via tensor_tensor_scan, final weighted
reduction via scalar_tensor_tensor accum_out).

Self-contained: hardcodes shapes from the problem spec.
"""

import numpy as np

P = 128                     # SBUF partitions == rays per batch
S = 128                     # samples per ray
RES = 128                   # voxel grid resolution
V = RES ** 3
NEAR, FAR = 0.05, 4.0
NUM_RAYS = 32768
N_CORES = 8
TABW = 28                   # 1 density + 27 SH coeffs per voxel row

# Spherical-harmonic basis constants (degree 2)
SH0 = 0.28209479177387814
SH1 = 0.4886025119029199
SH2 = 1.0925484305920792
SH3 = 0.31539156525252005
SH4 = 0.5462742152960396

_CACHE = {}


def build(n_rays, widths):
    """Bass/Tile program for one core: n_rays rays; per-batch window widths
    (rays are host-sorted by span so each batch gets a tight width)."""
    import concourse.bass as bass
    import concourse.bacc as bacc
    import concourse.mybir as mybir
    import concourse.tile as tile

    f32 = mybir.dt.float32
    i32 = mybir.dt.int32
    Alu = mybir.AluOpType
    Act = mybir.ActivationFunctionType

    assert n_rays % P == 0
    B = n_rays // P
    assert len(widths) == B
    DT = float(np.float32((FAR - NEAR) / (S - 1)))
    SWMAX = max(widths)

    nc = bacc.Bacc("TRN2", target_bir_lowering=False)
    rays_h = nc.dram_tensor("rays", [n_rays, 6], f32, kind="ExternalInput")
    tab_h = nc.dram_tensor("tab", [V, TABW], f32, kind="ExternalInput")
    tvec_h = nc.dram_tensor("tvec", [S + SWMAX, 1], f32, kind="ExternalInput")
    out_h = nc.dram_tensor("out", [n_rays, 3], f32, kind="ExternalOutput")

    def floorchain(pool, src, tagp):
        """floor(src) in f32, correct for truncating or rounding converters."""
        iv = pool.tile(list(src.shape), i32, tag=tagp + "_iv")
        nc.vector.tensor_copy(iv[:], src)
        fb = pool.tile(list(src.shape), f32, tag=tagp + "_fb")
        nc.vector.tensor_copy(fb[:], iv[:])
        gt = pool.tile(list(src.shape), f32, tag=tagp + "_gt")
        nc.vector.tensor_tensor(gt[:], fb[:], src, Alu.is_gt)
        nc.vector.tensor_tensor(fb[:], fb[:], gt[:], Alu.subtract)
        return fb

    with tile.TileContext(nc) as tc:
        with (
            tc.tile_pool(name="const", bufs=1) as cpool,
            tc.tile_pool(name="work", bufs=3) as wpool,
            tc.tile_pool(name="gath", bufs=4) as gpool,
        ):
            # ---- one-time setup ----
            rays_t = cpool.tile([P, B, 6], f32)
            nc.sync.dma_start(
                out=rays_t[:], in_=rays_h[:].rearrange("(b p) c -> p b c", p=P)
            )
            # basis [P, 9, B]
            bas_t = cpool.tile([P, 9, B], f32)
            t1 = cpool.tile([P, B], f32)
            xs = rays_t[:, :, 3]
            ys = rays_t[:, :, 4]
            zs = rays_t[:, :, 5]
            nc.vector.memset(bas_t[:, 0, :], SH0)
            nc.vector.tensor_scalar(bas_t[:, 1, :], ys, -SH1, None, Alu.mult)
            nc.vector.tensor_scalar(bas_t[:, 2, :], zs, SH1, None, Alu.mult)
            nc.vector.tensor_scalar(bas_t[:, 3, :], xs, -SH1, None, Alu.mult)
            nc.vector.scalar_tensor_tensor(bas_t[:, 4, :], xs, SH2, ys, Alu.mult, Alu.mult)
            nc.vector.scalar_tensor_tensor(bas_t[:, 5, :], ys, -SH2, zs, Alu.mult, Alu.mult)
            nc.vector.scalar_tensor_tensor(t1[:], zs, 3.0 * SH3, zs, Alu.mult, Alu.mult)
            nc.vector.tensor_scalar(bas_t[:, 6, :], t1[:], -SH3, None, Alu.add)
            nc.vector.scalar_tensor_tensor(bas_t[:, 7, :], xs, -SH2, zs, Alu.mult, Alu.mult)
            t2 = cpool.tile([P, B], f32)
            nc.vector.scalar_tensor_tensor(t2[:], xs, SH4, xs, Alu.mult, Alu.mult)
            nc.vector.scalar_tensor_tensor(bas_t[:, 8, :], ys, -SH4, ys, Alu.mult, Alu.mult)
            nc.vector.tensor_tensor(bas_t[:, 8, :], bas_t[:, 8, :], t2[:], Alu.add)

            # window math coefficients: u_c(s) = A_c*s + B_c (f32 approx,
            # used only for conservative window placement)
            A3 = cpool.tile([P, B, 3], f32)
            B3 = cpool.tile([P, B, 3], f32)
            o_view = rays_t[:, :, 0:3]
            d_view = rays_t[:, :, 3:6]
            nc.vector.tensor_scalar(A3[:], d_view, 64.0 * DT, None, Alu.mult)
            tmp3 = cpool.tile([P, B, 3], f32)
            nc.vector.tensor_scalar(tmp3[:], o_view, 64.0, 64.0, Alu.mult, Alu.add)
            nc.vector.scalar_tensor_tensor(
                B3[:], d_view, 64.0 * NEAR, tmp3[:], Alu.mult, Alu.add
            )

            res_t = cpool.tile([P, B, 3], f32)

            # ---- per ray-batch ----
            for b in range(B):
                SW = widths[b]
                # conservative per-ray window start a = clip(floor(enter)-2)
                negB = wpool.tile([P, 3], f32, tag="negB")
                nc.vector.tensor_scalar(negB[:], B3[:, b, :], -1.0, None, Alu.mult)
                rA = wpool.tile([P, 3], f32, tag="rA")
                nc.vector.reciprocal(rA[:], A3[:, b, :])
                sA = wpool.tile([P, 3], f32, tag="sA")
                nc.vector.tensor_tensor(sA[:], negB[:], rA[:], Alu.mult)
                sB = wpool.tile([P, 3], f32, tag="sB")
                nc.vector.tensor_scalar(sB[:], negB[:], 128.0, None, Alu.add)
                nc.vector.tensor_tensor(sB[:], sB[:], rA[:], Alu.mult)
                en = wpool.tile([P, 3], f32, tag="en")
                nc.vector.tensor_tensor(en[:], sA[:], sB[:], Alu.min)
                enm = wpool.tile([P, 1], f32, tag="enm")
                nc.vector.tensor_reduce(enm[:], en[:], mybir.AxisListType.X, Alu.max)
                nc.vector.tensor_scalar(enm[:], enm[:], -1.0, 0.0, Alu.add, Alu.max)
                nc.vector.tensor_scalar(enm[:], enm[:], float(S - 1), None, Alu.min)
                a_i = wpool.tile([P, 1], i32, tag="a_i")
                nc.vector.tensor_copy(a_i[:], enm[:])  # any-rounding: ±1 ok

                # exact t values for this ray's window: t_til[p,j]=tvec[a_p+j]
                t_til = wpool.tile([P, SW], f32, tag="t_til")
                nc.gpsimd.indirect_dma_start(
                    out=t_til[:],
                    out_offset=None,
                    in_=tvec_h[:],
                    in_offset=bass.IndirectOffsetOnAxis(ap=a_i[:], axis=0),
                )

                # u = ((o + t*d) + 1) * 64 with reference rounding order
                u3 = wpool.tile([P, 3, SW], f32, tag="u3")
                for c in range(3):
                    o_ap = rays_t[:, b, c:c + 1]
                    d_ap = rays_t[:, b, 3 + c:4 + c]
                    nc.vector.tensor_scalar(
                        u3[:, c, :], t_til[:], d_ap, o_ap, Alu.mult, Alu.add
                    )
                    nc.vector.tensor_scalar(
                        u3[:, c, :], u3[:, c, :], 1.0, 64.0, Alu.add, Alu.mult
                    )
                ff3 = floorchain(wpool, u3[:], "f3")
                cf3 = wpool.tile([P, 3, SW], f32, tag="cf3")
                nc.vector.tensor_scalar(cf3[:], ff3[:], 0.0, 127.0, Alu.max, Alu.min)
                meq = wpool.tile([P, 3, SW], f32, tag="meq")
                nc.vector.tensor_tensor(meq[:], ff3[:], cf3[:], Alu.is_equal)
                mask = wpool.tile([P, SW], f32, tag="mask")
                nc.vector.tensor_tensor(mask[:], meq[:, 0, :], meq[:, 1, :], Alu.mult)
                nc.vector.tensor_tensor(mask[:], mask[:], meq[:, 2, :], Alu.mult)
                flatf = wpool.tile([P, SW], f32, tag="flatf")
                nc.vector.scalar_tensor_tensor(
                    flatf[:], cf3[:, 0, :], 128.0, cf3[:, 1, :], Alu.mult, Alu.add
                )
                nc.vector.scalar_tensor_tensor(
                    flatf[:], flatf[:], 128.0, cf3[:, 2, :], Alu.mult, Alu.add
                )
                idx = wpool.tile([P, SW], i32, tag="idx")
                nc.vector.tensor_copy(idx[:], flatf[:])

                # gather [P, SW, 28] voxel rows (one 128-row call per column)
                gath = gpool.tile([P, SW, TABW], f32, tag="gath")
                for s in range(SW):
                    nc.gpsimd.indirect_dma_start(
                        out=gath[:, s, :],
                        out_offset=None,
                        in_=tab_h[:],
                        in_offset=bass.IndirectOffsetOnAxis(ap=idx[:, s:s + 1], axis=0),
                    )

                # density -> q = exp(-dt*mask*exp(den))
                e_t = wpool.tile([P, SW], f32, tag="e_t")
                nc.scalar.activation(e_t[:], gath[:, :, 0], Act.Exp)
                nege = wpool.tile([P, SW], f32, tag="nege")
                nc.vector.scalar_tensor_tensor(
                    nege[:], e_t[:], -DT, mask[:], Alu.mult, Alu.mult
                )
                q_t = wpool.tile([P, SW], f32, tag="q_t")
                nc.scalar.activation(q_t[:], nege[:], Act.Exp)
                T_t = wpool.tile([P, SW + 1], f32, tag="T_t")
                nc.vector.memset(T_t[:, 0:1], 1.0)
                nc.vector.tensor_tensor_scan(
                    T_t[:, 1:SW + 1], q_t[:], q_t[:], 1.0, Alu.mult, Alu.bypass
                )
                w_t = wpool.tile([P, SW], f32, tag="w_t")
                nc.vector.tensor_tensor(
                    w_t[:], T_t[:, 0:SW], T_t[:, 1:SW + 1], Alu.subtract
                )

                # SH shading
                acc = wpool.tile([P, SW, 3], f32, tag="acc")
                nc.vector.tensor_scalar(acc[:], gath[:, :, 1:4], SH0, None, Alu.mult)
                for k in range(1, 9):
                    nc.vector.scalar_tensor_tensor(
                        acc[:],
                        gath[:, :, 1 + 3 * k:4 + 3 * k],
                        bas_t[:, k, b:b + 1],
                        acc[:],
                        Alu.mult,
                        Alu.add,
                    )
                # sigmoid via exp + reciprocal (single ACT table)
                rgbs = wpool.tile([P, SW, 3], f32, tag="rgbs")
                nc.scalar.activation(rgbs[:], acc[:], Act.Exp, scale=-1.0)
                nc.vector.tensor_scalar(rgbs[:], rgbs[:], 1.0, None, Alu.add)
                nc.vector.reciprocal(rgbs[:], rgbs[:])

                # weighted reduction over window samples
                scr = wpool.tile([P, 3, SW], f32, tag="scr")
                for c in range(3):
                    nc.vector.scalar_tensor_tensor(
                        scr[:, c, :],
                        rgbs[:, :, c],
                        1.0,
                        w_t[:],
                        Alu.mult,
                        Alu.mult,
                        accum_out=res_t[:, b, c:c + 1],
                    )

            nc.sync.dma_start(
                out=out_h[:].rearrange("(b p) c -> p b c", p=P), in_=res_t[:]
            )

    nc.compile()
    return nc


def _plan(rays_o, rays_d):
    """Sort rays by in-bounds span and assign them to cores/batches so every
    core's batch j can use the same (tight) window width.

    Returns (core_ray_ids: list of N_CORES index arrays, widths: tuple)."""
    o = np.asarray(rays_o, np.float64)
    d = np.asarray(rays_d, np.float64)
    t = np.linspace(NEAR, FAR, S)
    u = (o[:, None, :] + d[:, None, :] * t[None, :, None] + 1.0) * 64.0
    inb = np.all((u >= 0) & (u < 128), axis=-1)  # [R, S] approx in-bounds
    any_r = inb.any(axis=1)
    first = np.where(any_r, inb.argmax(axis=1), 0)
    last = np.where(any_r, S - 1 - inb[:, ::-1].argmax(axis=1), 0)
    span = np.where(any_r, last - first + 1, 1).astype(np.int64)
    order = np.argsort(-span, kind="stable")
    n = len(order)
    b_total = n // P
    bc = b_total // N_CORES
    ss = span[order]
    widths = tuple(
        int(min(S, max(8, ss[(j * N_CORES) * P] + 4))) for j in range(bc)
    )
    core_rays = [[] for _ in range(N_CORES)]
    for g in range(b_total):
        core_rays[g % N_CORES].append(order[g * P:(g + 1) * P])
    core_rays = [np.concatenate(x) for x in core_rays]
    return core_rays, widths


def _host_prep(rays_o, rays_d, voxel_density, voxel_sh, s_w):
    import jax.numpy as jnp

    tab = np.empty((V, TABW), dtype=np.float32)
    tab[:, 0] = np.asarray(voxel_density, dtype=np.float32)
    tab[:, 1:] = np.asarray(voxel_sh, dtype=np.float32)
    rays = np.concatenate(
        [np.asarray(rays_o, np.float32), np.asarray(rays_d, np.float32)], axis=1
    )  # [NUM_RAYS, 6]
    tvec = np.full((S + s_w, 1), 1e9, dtype=np.float32)
    tvec[:S, 0] = np.asarray(
        jnp.linspace(NEAR, FAR, S, dtype=jnp.float32), dtype=np.float32
    )
    return rays, tab, tvec


def kernel(rays_o, rays_d, voxel_density, voxel_sh):
    from concourse.bass_utils import run_bass_kernel_spmd

    n_rays = rays_o.shape[0]
    per_core = n_rays // N_CORES
    core_rays, widths = _plan(rays_o, rays_d)
    rays, tab, tvec = _host_prep(rays_o, rays_d, voxel_density, voxel_sh,
                                 max(widths))

    key = ("nc", per_core, widths)
    if key not in _CACHE:
        _CACHE[key] = build(per_core, widths)
    nc = _CACHE[key]

    in_maps = [
        {
            "rays": np.ascontiguousarray(rays[core_rays[c]]),
            "tab": tab,
            "tvec": tvec,
        }
        for c in range(N_CORES)
    ]
    res = run_bass_kernel_spmd(nc, in_maps, core_ids=list(range(N_CORES)))
    out = np.empty((n_rays, 3), np.float32)
    for c in range(N_CORES):
        out[core_rays[c]] = res.results[c]["out"]
    return out


# revision 12
# speedup vs baseline: 1.8631x; 1.0287x over previous
"""SVRaster (sparse-voxel NeRF-style raymarcher) for 8x Trainium2 NeuronCores.

Data-parallel over rays: each of the 8 cores renders NUM_RAYS/8 rays against
the full (replicated) voxel tables.  Per ray, only a per-ray window of S_w
samples (covering the ray/scene-box intersection) is processed: the voxel
index is computed on-chip, density+SH coefficients are fetched with one
128-row indirect DMA per sample column, and SH shading + alpha compositing
run on DVE/ACT (compositing cumprod via tensor_tensor_scan, final weighted
reduction via scalar_tensor_tensor accum_out).

Self-contained: hardcodes shapes from the problem spec.
"""

import numpy as np

P = 128                     # SBUF partitions == rays per batch
S = 128                     # samples per ray
RES = 128                   # voxel grid resolution
V = RES ** 3
NEAR, FAR = 0.05, 4.0
NUM_RAYS = 32768
N_CORES = 8
TABW = 28                   # 1 density + 27 SH coeffs per voxel row

# Spherical-harmonic basis constants (degree 2)
SH0 = 0.28209479177387814
SH1 = 0.4886025119029199
SH2 = 1.0925484305920792
SH3 = 0.31539156525252005
SH4 = 0.5462742152960396

_CACHE = {}


def build(n_rays, widths):
    """Bass/Tile program for one core: n_rays rays; per-batch window widths
    (rays are host-sorted by span so each batch gets a tight width)."""
    import concourse.bass as bass
    import concourse.bacc as bacc
    import concourse.mybir as mybir
    import concourse.tile as tile

    f32 = mybir.dt.float32
    i32 = mybir.dt.int32
    Alu = mybir.AluOpType
    Act = mybir.ActivationFunctionType

    assert n_rays % P == 0
    B = n_rays // P
    assert len(widths) == B
    DT = float(np.float32((FAR - NEAR) / (S - 1)))
    SWMAX = max(widths)

    nc = bacc.Bacc("TRN2", target_bir_lowering=False)
    rays_h = nc.dram_tensor("rays", [n_rays, 6], f32, kind="ExternalInput")
    tab_h = nc.dram_tensor("tab", [V, TABW], f32, kind="ExternalInput")
    tvec_h = nc.dram_tensor("tvec", [S + SWMAX, 1], f32, kind="ExternalInput")
    out_h = nc.dram_tensor("out", [n_rays, 3], f32, kind="ExternalOutput")

    def floorchain(pool, src, tagp):
        """floor(src) in f32, correct for truncating or rounding converters."""
        iv = pool.tile(list(src.shape), i32, tag=tagp + "_iv")
        nc.vector.tensor_copy(iv[:], src)
        fb = pool.tile(list(src.shape), f32, tag=tagp + "_fb")
        nc.vector.tensor_copy(fb[:], iv[:])
        gt = pool.tile(list(src.shape), f32, tag=tagp + "_gt")
        nc.vector.tensor_tensor(gt[:], fb[:], src, Alu.is_gt)
        nc.vector.tensor_tensor(fb[:], fb[:], gt[:], Alu.subtract)
        return fb

    with tile.TileContext(nc) as tc:
        with (
            tc.tile_pool(name="const", bufs=1) as cpool,
            tc.tile_pool(name="work", bufs=4) as wpool,
            tc.tile_pool(name="gath", bufs=6) as gpool,
        ):
            # ---- one-time setup ----
            rays_t = cpool.tile([P, B, 6], f32)
            nc.sync.dma_start(
                out=rays_t[:], in_=rays_h[:].rearrange("(b p) c -> p b c", p=P)
            )
            # basis [P, 9, B]
            bas_t = cpool.tile([P, 9, B], f32)
            t1 = cpool.tile([P, B], f32)
            xs = rays_t[:, :, 3]
            ys = rays_t[:, :, 4]
            zs = rays_t[:, :, 5]
            nc.vector.memset(bas_t[:, 0, :], SH0)
            nc.vector.tensor_scalar(bas_t[:, 1, :], ys, -SH1, None, Alu.mult)
            nc.vector.tensor_scalar(bas_t[:, 2, :], zs, SH1, None, Alu.mult)
            nc.vector.tensor_scalar(bas_t[:, 3, :], xs, -SH1, None, Alu.mult)
            nc.vector.scalar_tensor_tensor(bas_t[:, 4, :], xs, SH2, ys, Alu.mult, Alu.mult)
            nc.vector.scalar_tensor_tensor(bas_t[:, 5, :], ys, -SH2, zs, Alu.mult, Alu.mult)
            nc.vector.scalar_tensor_tensor(t1[:], zs, 3.0 * SH3, zs, Alu.mult, Alu.mult)
            nc.vector.tensor_scalar(bas_t[:, 6, :], t1[:], -SH3, None, Alu.add)
            nc.vector.scalar_tensor_tensor(bas_t[:, 7, :], xs, -SH2, zs, Alu.mult, Alu.mult)
            t2 = cpool.tile([P, B], f32)
            nc.vector.scalar_tensor_tensor(t2[:], xs, SH4, xs, Alu.mult, Alu.mult)
            nc.vector.scalar_tensor_tensor(bas_t[:, 8, :], ys, -SH4, ys, Alu.mult, Alu.mult)
            nc.vector.tensor_tensor(bas_t[:, 8, :], bas_t[:, 8, :], t2[:], Alu.add)

            # window math coefficients: u_c(s) = A_c*s + B_c (f32 approx,
            # used only for conservative window placement)
            A3 = cpool.tile([P, B, 3], f32)
            B3 = cpool.tile([P, B, 3], f32)
            o_view = rays_t[:, :, 0:3]
            d_view = rays_t[:, :, 3:6]
            nc.vector.tensor_scalar(A3[:], d_view, 64.0 * DT, None, Alu.mult)
            tmp3 = cpool.tile([P, B, 3], f32)
            nc.vector.tensor_scalar(tmp3[:], o_view, 64.0, 64.0, Alu.mult, Alu.add)
            nc.vector.scalar_tensor_tensor(
                B3[:], d_view, 64.0 * NEAR, tmp3[:], Alu.mult, Alu.add
            )

            res_t = cpool.tile([P, B, 3], f32)

            # ---- prologue: per-ray window starts for ALL batches at once ----
            # a = clip(floor(box-enter sample) - 1, 0, S-1), conservative
            negB = cpool.tile([P, B, 3], f32)
            nc.vector.tensor_scalar(negB[:], B3[:], -1.0, None, Alu.mult)
            rA = cpool.tile([P, B, 3], f32)
            nc.vector.reciprocal(rA[:], A3[:])
            sA = cpool.tile([P, B, 3], f32)
            nc.vector.tensor_tensor(sA[:], negB[:], rA[:], Alu.mult)
            sB = cpool.tile([P, B, 3], f32)
            nc.vector.tensor_scalar(sB[:], negB[:], 128.0, None, Alu.add)
            nc.vector.tensor_tensor(sB[:], sB[:], rA[:], Alu.mult)
            en = cpool.tile([P, B, 3], f32)
            nc.vector.tensor_tensor(en[:], sA[:], sB[:], Alu.min)
            enm = cpool.tile([P, B], f32)
            nc.vector.tensor_reduce(enm[:], en[:], mybir.AxisListType.X, Alu.max)
            nc.vector.tensor_scalar(enm[:], enm[:], -1.0, 0.0, Alu.add, Alu.max)
            nc.vector.tensor_scalar(enm[:], enm[:], float(S - 1), None, Alu.min)
            a_i = cpool.tile([P, B], i32)
            nc.vector.tensor_copy(a_i[:], enm[:])  # any-rounding: ±1 ok

            # exact t values per window: t_all[p,b,j] = tvec[a[p,b]+j]
            t_all = cpool.tile([P, B, max(widths)], f32)
            for b in range(B):
                nc.gpsimd.indirect_dma_start(
                    out=t_all[:, b, 0:widths[b]],
                    out_offset=None,
                    in_=tvec_h[:],
                    in_offset=bass.IndirectOffsetOnAxis(ap=a_i[:, b:b + 1], axis=0),
                )

            # ---- per ray-batch ----
            for b in range(B):
                SW = widths[b]
                t_til = t_all[:, b, 0:SW]

                # u = ((o + t*d) + 1) * 64 with reference rounding order
                u3 = wpool.tile([P, 3, SW], f32, tag="u3")
                for c in range(3):
                    o_ap = rays_t[:, b, c:c + 1]
                    d_ap = rays_t[:, b, 3 + c:4 + c]
                    nc.vector.tensor_scalar(
                        u3[:, c, :], t_til, d_ap, o_ap, Alu.mult, Alu.add
                    )
                    nc.vector.tensor_scalar(
                        u3[:, c, :], u3[:, c, :], 1.0, 64.0, Alu.add, Alu.mult
                    )
                ff3 = floorchain(wpool, u3[:], "f3")
                cf3 = wpool.tile([P, 3, SW], f32, tag="cf3")
                nc.vector.tensor_scalar(cf3[:], ff3[:], 0.0, 127.0, Alu.max, Alu.min)
                meq = wpool.tile([P, 3, SW], f32, tag="meq")
                nc.vector.tensor_tensor(meq[:], ff3[:], cf3[:], Alu.is_equal)
                mask = wpool.tile([P, SW], f32, tag="mask")
                nc.vector.tensor_tensor(mask[:], meq[:, 0, :], meq[:, 1, :], Alu.mult)
                nc.vector.tensor_tensor(mask[:], mask[:], meq[:, 2, :], Alu.mult)
                flatf = wpool.tile([P, SW], f32, tag="flatf")
                nc.vector.scalar_tensor_tensor(
                    flatf[:], cf3[:, 0, :], 128.0, cf3[:, 1, :], Alu.mult, Alu.add
                )
                nc.vector.scalar_tensor_tensor(
                    flatf[:], flatf[:], 128.0, cf3[:, 2, :], Alu.mult, Alu.add
                )
                idx = wpool.tile([P, SW], i32, tag="idx")
                nc.vector.tensor_copy(idx[:], flatf[:])

                # gather [P, SW, 28] voxel rows (one 128-row call per column)
                gath = gpool.tile([P, SW, TABW], f32, tag="gath")
                for s in range(SW):
                    nc.gpsimd.indirect_dma_start(
                        out=gath[:, s, :],
                        out_offset=None,
                        in_=tab_h[:],
                        in_offset=bass.IndirectOffsetOnAxis(ap=idx[:, s:s + 1], axis=0),
                    )

                # density -> q = exp(-dt*mask*exp(den))
                e_t = wpool.tile([P, SW], f32, tag="e_t")
                nc.scalar.activation(e_t[:], gath[:, :, 0], Act.Exp)
                nege = wpool.tile([P, SW], f32, tag="nege")
                nc.vector.scalar_tensor_tensor(
                    nege[:], e_t[:], -DT, mask[:], Alu.mult, Alu.mult
                )
                q_t = wpool.tile([P, SW], f32, tag="q_t")
                nc.scalar.activation(q_t[:], nege[:], Act.Exp)
                T_t = wpool.tile([P, SW + 1], f32, tag="T_t")
                nc.vector.memset(T_t[:, 0:1], 1.0)
                nc.vector.tensor_tensor_scan(
                    T_t[:, 1:SW + 1], q_t[:], q_t[:], 1.0, Alu.mult, Alu.bypass
                )
                w_t = wpool.tile([P, SW], f32, tag="w_t")
                nc.vector.tensor_tensor(
                    w_t[:], T_t[:, 0:SW], T_t[:, 1:SW + 1], Alu.subtract
                )

                # SH shading
                acc = wpool.tile([P, SW, 3], f32, tag="acc")
                nc.vector.tensor_scalar(acc[:], gath[:, :, 1:4], SH0, None, Alu.mult)
                for k in range(1, 9):
                    nc.vector.scalar_tensor_tensor(
                        acc[:],
                        gath[:, :, 1 + 3 * k:4 + 3 * k],
                        bas_t[:, k, b:b + 1],
                        acc[:],
                        Alu.mult,
                        Alu.add,
                    )
                # sigmoid via exp + reciprocal (single ACT table)
                rgbs = wpool.tile([P, SW, 3], f32, tag="rgbs")
                nc.scalar.activation(rgbs[:], acc[:], Act.Exp, scale=-1.0)
                nc.vector.tensor_scalar(rgbs[:], rgbs[:], 1.0, None, Alu.add)
                nc.vector.reciprocal(rgbs[:], rgbs[:])

                # weighted reduction over window samples
                scr = wpool.tile([P, 3, SW], f32, tag="scr")
                for c in range(3):
                    nc.vector.scalar_tensor_tensor(
                        scr[:, c, :],
                        rgbs[:, :, c],
                        1.0,
                        w_t[:],
                        Alu.mult,
                        Alu.mult,
                        accum_out=res_t[:, b, c:c + 1],
                    )

            nc.sync.dma_start(
                out=out_h[:].rearrange("(b p) c -> p b c", p=P), in_=res_t[:]
            )

    nc.compile()
    return nc


def _plan(rays_o, rays_d):
    """Sort rays by in-bounds span and assign them to cores/batches so every
    core's batch j can use the same (tight) window width.

    Returns (core_ray_ids: list of N_CORES index arrays, widths: tuple)."""
    o = np.asarray(rays_o, np.float64)
    d = np.asarray(rays_d, np.float64)
    t = np.linspace(NEAR, FAR, S)
    u = (o[:, None, :] + d[:, None, :] * t[None, :, None] + 1.0) * 64.0
    inb = np.all((u >= 0) & (u < 128), axis=-1)  # [R, S] approx in-bounds
    any_r = inb.any(axis=1)
    first = np.where(any_r, inb.argmax(axis=1), 0)
    last = np.where(any_r, S - 1 - inb[:, ::-1].argmax(axis=1), 0)
    span = np.where(any_r, last - first + 1, 1).astype(np.int64)
    order = np.argsort(-span, kind="stable")
    n = len(order)
    b_total = n // P
    bc = b_total // N_CORES
    ss = span[order]
    widths = tuple(
        int(min(S, max(8, ss[(j * N_CORES) * P] + 4))) for j in range(bc)
    )
    core_rays = [[] for _ in range(N_CORES)]
    for g in range(b_total):
        core_rays[g % N_CORES].append(order[g * P:(g + 1) * P])
    core_rays = [np.concatenate(x) for x in core_rays]
    return core_rays, widths


def _host_prep(rays_o, rays_d, voxel_density, voxel_sh, s_w):
    import jax.numpy as jnp

    tab = np.empty((V, TABW), dtype=np.float32)
    tab[:, 0] = np.asarray(voxel_density, dtype=np.float32)
    tab[:, 1:] = np.asarray(voxel_sh, dtype=np.float32)
    rays = np.concatenate(
        [np.asarray(rays_o, np.float32), np.asarray(rays_d, np.float32)], axis=1
    )  # [NUM_RAYS, 6]
    tvec = np.full((S + s_w, 1), 1e9, dtype=np.float32)
    tvec[:S, 0] = np.asarray(
        jnp.linspace(NEAR, FAR, S, dtype=jnp.float32), dtype=np.float32
    )
    return rays, tab, tvec


def kernel(rays_o, rays_d, voxel_density, voxel_sh):
    from concourse.bass_utils import run_bass_kernel_spmd

    n_rays = rays_o.shape[0]
    per_core = n_rays // N_CORES
    core_rays, widths = _plan(rays_o, rays_d)
    rays, tab, tvec = _host_prep(rays_o, rays_d, voxel_density, voxel_sh,
                                 max(widths))

    key = ("nc", per_core, widths)
    if key not in _CACHE:
        _CACHE[key] = build(per_core, widths)
    nc = _CACHE[key]

    in_maps = [
        {
            "rays": np.ascontiguousarray(rays[core_rays[c]]),
            "tab": tab,
            "tvec": tvec,
        }
        for c in range(N_CORES)
    ]
    res = run_bass_kernel_spmd(nc, in_maps, core_ids=list(range(N_CORES)))
    out = np.empty((n_rays, 3), np.float32)
    for c in range(N_CORES):
        out[core_rays[c]] = res.results[c]["out"]
    return out


# revision 13
# speedup vs baseline: 1.8947x; 1.0170x over previous
"""SVRaster (sparse-voxel NeRF-style raymarcher) for 8x Trainium2 NeuronCores.

Data-parallel over rays: each of the 8 cores renders NUM_RAYS/8 rays against
the full (replicated) voxel tables.  Per ray, only a per-ray window of S_w
samples (covering the ray/scene-box intersection) is processed: the voxel
index is computed on-chip, density+SH coefficients are fetched with one
128-row indirect DMA per sample column, and SH shading + alpha compositing
run on DVE/ACT (compositing cumprod via tensor_tensor_scan, final weighted
reduction via scalar_tensor_tensor accum_out).

Self-contained: hardcodes shapes from the problem spec.
"""

import numpy as np

P = 128                     # SBUF partitions == rays per batch
S = 128                     # samples per ray
RES = 128                   # voxel grid resolution
V = RES ** 3
NEAR, FAR = 0.05, 4.0
NUM_RAYS = 32768
N_CORES = 8
TABW = 28                   # 1 density + 27 SH coeffs per voxel row

# Spherical-harmonic basis constants (degree 2)
SH0 = 0.28209479177387814
SH1 = 0.4886025119029199
SH2 = 1.0925484305920792
SH3 = 0.31539156525252005
SH4 = 0.5462742152960396

_CACHE = {}


def build(n_rays, widths):
    """Bass/Tile program for one core: n_rays rays; per-batch window widths
    (rays are host-sorted by span so each batch gets a tight width)."""
    import concourse.bass as bass
    import concourse.bacc as bacc
    import concourse.mybir as mybir
    import concourse.tile as tile

    f32 = mybir.dt.float32
    i32 = mybir.dt.int32
    Alu = mybir.AluOpType
    Act = mybir.ActivationFunctionType

    assert n_rays % P == 0
    B = n_rays // P
    assert len(widths) == B
    DT = float(np.float32((FAR - NEAR) / (S - 1)))
    SWMAX = max(widths)

    nc = bacc.Bacc("TRN2", target_bir_lowering=False)
    rays_h = nc.dram_tensor("rays", [n_rays, 6], f32, kind="ExternalInput")
    tab_h = nc.dram_tensor("tab", [V, TABW], f32, kind="ExternalInput")
    tvec_h = nc.dram_tensor("tvec", [S + SWMAX, 1], f32, kind="ExternalInput")
    out_h = nc.dram_tensor("out", [n_rays, 3], f32, kind="ExternalOutput")

    def floorchain(pool, src, tagp):
        """floor(src) in f32, correct for truncating or rounding converters."""
        iv = pool.tile(list(src.shape), i32, tag=tagp + "_iv")
        nc.vector.tensor_copy(iv[:], src)
        fb = pool.tile(list(src.shape), f32, tag=tagp + "_fb")
        nc.vector.tensor_copy(fb[:], iv[:])
        gt = pool.tile(list(src.shape), f32, tag=tagp + "_gt")
        nc.vector.tensor_tensor(gt[:], fb[:], src, Alu.is_gt)
        nc.vector.tensor_tensor(fb[:], fb[:], gt[:], Alu.subtract)
        return fb

    with tile.TileContext(nc) as tc:
        with (
            tc.tile_pool(name="const", bufs=1) as cpool,
            tc.tile_pool(name="work", bufs=4) as wpool,
            tc.tile_pool(name="gath", bufs=6) as gpool,
        ):
            # ---- one-time setup ----
            rays_t = cpool.tile([P, B, 6], f32)
            nc.sync.dma_start(
                out=rays_t[:], in_=rays_h[:].rearrange("(b p) c -> p b c", p=P)
            )
            # basis [P, 9, B]
            bas_t = cpool.tile([P, 9, B], f32)
            t1 = cpool.tile([P, B], f32)
            xs = rays_t[:, :, 3]
            ys = rays_t[:, :, 4]
            zs = rays_t[:, :, 5]
            nc.vector.memset(bas_t[:, 0, :], SH0)
            nc.vector.tensor_scalar(bas_t[:, 1, :], ys, -SH1, None, Alu.mult)
            nc.vector.tensor_scalar(bas_t[:, 2, :], zs, SH1, None, Alu.mult)
            nc.vector.tensor_scalar(bas_t[:, 3, :], xs, -SH1, None, Alu.mult)
            nc.vector.scalar_tensor_tensor(bas_t[:, 4, :], xs, SH2, ys, Alu.mult, Alu.mult)
            nc.vector.scalar_tensor_tensor(bas_t[:, 5, :], ys, -SH2, zs, Alu.mult, Alu.mult)
            nc.vector.scalar_tensor_tensor(t1[:], zs, 3.0 * SH3, zs, Alu.mult, Alu.mult)
            nc.vector.tensor_scalar(bas_t[:, 6, :], t1[:], -SH3, None, Alu.add)
            nc.vector.scalar_tensor_tensor(bas_t[:, 7, :], xs, -SH2, zs, Alu.mult, Alu.mult)
            t2 = cpool.tile([P, B], f32)
            nc.vector.scalar_tensor_tensor(t2[:], xs, SH4, xs, Alu.mult, Alu.mult)
            nc.vector.scalar_tensor_tensor(bas_t[:, 8, :], ys, -SH4, ys, Alu.mult, Alu.mult)
            nc.vector.tensor_tensor(bas_t[:, 8, :], bas_t[:, 8, :], t2[:], Alu.add)

            # window math coefficients: u_c(s) = A_c*s + B_c (f32 approx,
            # used only for conservative window placement)
            A3 = cpool.tile([P, B, 3], f32)
            B3 = cpool.tile([P, B, 3], f32)
            o_view = rays_t[:, :, 0:3]
            d_view = rays_t[:, :, 3:6]
            nc.vector.tensor_scalar(A3[:], d_view, 64.0 * DT, None, Alu.mult)
            tmp3 = cpool.tile([P, B, 3], f32)
            nc.vector.tensor_scalar(tmp3[:], o_view, 64.0, 64.0, Alu.mult, Alu.add)
            nc.vector.scalar_tensor_tensor(
                B3[:], d_view, 64.0 * NEAR, tmp3[:], Alu.mult, Alu.add
            )

            res_t = cpool.tile([P, B, 3], f32)

            # ---- prologue: per-ray window starts for ALL batches at once ----
            # a = clip(floor(box-enter sample) - 1, 0, S-1), conservative
            negB = cpool.tile([P, B, 3], f32)
            nc.vector.tensor_scalar(negB[:], B3[:], -1.0, None, Alu.mult)
            rA = cpool.tile([P, B, 3], f32)
            nc.vector.reciprocal(rA[:], A3[:])
            sA = cpool.tile([P, B, 3], f32)
            nc.vector.tensor_tensor(sA[:], negB[:], rA[:], Alu.mult)
            sB = cpool.tile([P, B, 3], f32)
            nc.vector.tensor_scalar(sB[:], negB[:], 128.0, None, Alu.add)
            nc.vector.tensor_tensor(sB[:], sB[:], rA[:], Alu.mult)
            en = cpool.tile([P, B, 3], f32)
            nc.vector.tensor_tensor(en[:], sA[:], sB[:], Alu.min)
            enm = cpool.tile([P, B], f32)
            nc.vector.tensor_reduce(enm[:], en[:], mybir.AxisListType.X, Alu.max)
            nc.vector.tensor_scalar(enm[:], enm[:], -1.0, 0.0, Alu.add, Alu.max)
            nc.vector.tensor_scalar(enm[:], enm[:], float(S - 1), None, Alu.min)
            a_i = cpool.tile([P, B], i32)
            nc.vector.tensor_copy(a_i[:], enm[:])  # any-rounding: ±1 ok

            # exact t values per window: t_all[p,b,j] = tvec[a[p,b]+j]
            t_all = cpool.tile([P, B, max(widths)], f32)
            for b in range(B):
                nc.gpsimd.indirect_dma_start(
                    out=t_all[:, b, 0:widths[b]],
                    out_offset=None,
                    in_=tvec_h[:],
                    in_offset=bass.IndirectOffsetOnAxis(ap=a_i[:, b:b + 1], axis=0),
                )

            # ---- per ray-batch ----
            for b in range(B):
                SW = widths[b]
                t_til = t_all[:, b, 0:SW]

                # u = ((o + t*d) + 1) * 64 with reference rounding order
                u3 = wpool.tile([P, 3, SW], f32, tag="u3")
                for c in range(3):
                    o_ap = rays_t[:, b, c:c + 1]
                    d_ap = rays_t[:, b, 3 + c:4 + c]
                    nc.vector.tensor_scalar(
                        u3[:, c, :], t_til, d_ap, o_ap, Alu.mult, Alu.add
                    )
                    nc.vector.tensor_scalar(
                        u3[:, c, :], u3[:, c, :], 1.0, 64.0, Alu.add, Alu.mult
                    )
                ff3 = floorchain(wpool, u3[:], "f3")
                cf3 = wpool.tile([P, 3, SW], f32, tag="cf3")
                nc.vector.tensor_scalar(cf3[:], ff3[:], 0.0, 127.0, Alu.max, Alu.min)
                meq = wpool.tile([P, 3, SW], f32, tag="meq")
                nc.vector.tensor_tensor(meq[:], ff3[:], cf3[:], Alu.is_equal)
                mask = wpool.tile([P, SW], f32, tag="mask")
                nc.vector.tensor_tensor(mask[:], meq[:, 0, :], meq[:, 1, :], Alu.mult)
                nc.vector.tensor_tensor(mask[:], mask[:], meq[:, 2, :], Alu.mult)
                flatf = wpool.tile([P, SW], f32, tag="flatf")
                nc.vector.scalar_tensor_tensor(
                    flatf[:], cf3[:, 0, :], 128.0, cf3[:, 1, :], Alu.mult, Alu.add
                )
                nc.vector.scalar_tensor_tensor(
                    flatf[:], flatf[:], 128.0, cf3[:, 2, :], Alu.mult, Alu.add
                )
                idx = wpool.tile([P, SW], i32, tag="idx")
                nc.vector.tensor_copy(idx[:], flatf[:])

                # gather [P, SW, 28] voxel rows (one 128-row call per column)
                gath = gpool.tile([P, SW, TABW], f32, tag="gath")
                for s in range(SW):
                    nc.gpsimd.indirect_dma_start(
                        out=gath[:, s, :],
                        out_offset=None,
                        in_=tab_h[:],
                        in_offset=bass.IndirectOffsetOnAxis(ap=idx[:, s:s + 1], axis=0),
                    )

                # density -> q = exp(-dt*mask*exp(den))
                e_t = wpool.tile([P, SW], f32, tag="e_t")
                nc.scalar.activation(e_t[:], gath[:, :, 0], Act.Exp)
                nege = wpool.tile([P, SW], f32, tag="nege")
                nc.vector.scalar_tensor_tensor(
                    nege[:], e_t[:], -DT, mask[:], Alu.mult, Alu.mult
                )
                q_t = wpool.tile([P, SW], f32, tag="q_t")
                nc.scalar.activation(q_t[:], nege[:], Act.Exp)
                T_t = wpool.tile([P, SW + 1], f32, tag="T_t")
                nc.vector.memset(T_t[:, 0:1], 1.0)
                nc.vector.tensor_tensor_scan(
                    T_t[:, 1:SW + 1], q_t[:], q_t[:], 1.0, Alu.mult, Alu.bypass
                )
                w_t = wpool.tile([P, SW], f32, tag="w_t")
                nc.vector.tensor_tensor(
                    w_t[:], T_t[:, 0:SW], T_t[:, 1:SW + 1], Alu.subtract
                )

                # SH shading
                acc = wpool.tile([P, SW, 3], f32, tag="acc")
                nc.vector.tensor_scalar(acc[:], gath[:, :, 1:4], SH0, None, Alu.mult)
                for k in range(1, 9):
                    nc.vector.scalar_tensor_tensor(
                        acc[:],
                        gath[:, :, 1 + 3 * k:4 + 3 * k],
                        bas_t[:, k, b:b + 1],
                        acc[:],
                        Alu.mult,
                        Alu.add,
                    )
                # sigmoid via exp + reciprocal (single ACT table)
                rgbs = wpool.tile([P, SW, 3], f32, tag="rgbs")
                nc.scalar.activation(rgbs[:], acc[:], Act.Exp, scale=-1.0)
                nc.vector.tensor_scalar(rgbs[:], rgbs[:], 1.0, None, Alu.add)
                nc.vector.reciprocal(rgbs[:], rgbs[:])

                # weighted reduction over window samples
                scr = wpool.tile([P, 3, SW], f32, tag="scr")
                for c in range(3):
                    nc.vector.scalar_tensor_tensor(
                        scr[:, c, :],
                        rgbs[:, :, c],
                        1.0,
                        w_t[:],
                        Alu.mult,
                        Alu.mult,
                        accum_out=res_t[:, b, c:c + 1],
                    )

            nc.sync.dma_start(
                out=out_h[:].rearrange("(b p) c -> p b c", p=P), in_=res_t[:]
            )

    nc.compile()
    return nc


def _plan(rays_o, rays_d):
    """Sort rays by in-bounds span and assign them to cores/batches so every
    core's batch j can use the same (tight) window width.

    Returns (core_ray_ids: list of N_CORES index arrays, widths: tuple)."""
    o = np.asarray(rays_o, np.float64)
    d = np.asarray(rays_d, np.float64)
    t = np.linspace(NEAR, FAR, S)
    u = (o[:, None, :] + d[:, None, :] * t[None, :, None] + 1.0) * 64.0
    inb = np.all((u >= 0) & (u < 128), axis=-1)  # [R, S] approx in-bounds
    any_r = inb.any(axis=1)
    first = np.where(any_r, inb.argmax(axis=1), 0)
    last = np.where(any_r, S - 1 - inb[:, ::-1].argmax(axis=1), 0)
    span = np.where(any_r, last - first + 1, 1).astype(np.int64)
    order = np.argsort(-span, kind="stable")
    n = len(order)
    b_total = n // P
    bc = b_total // N_CORES
    ss = span[order]
    widths = tuple(
        int(min(S, max(8, ss[(j * N_CORES) * P] + 3))) for j in range(bc)
    )
    core_rays = [[] for _ in range(N_CORES)]
    for g in range(b_total):
        core_rays[g % N_CORES].append(order[g * P:(g + 1) * P])
    core_rays = [np.concatenate(x) for x in core_rays]
    return core_rays, widths


def _host_prep(rays_o, rays_d, voxel_density, voxel_sh, s_w):
    import jax.numpy as jnp

    tab = np.empty((V, TABW), dtype=np.float32)
    tab[:, 0] = np.asarray(voxel_density, dtype=np.float32)
    tab[:, 1:] = np.asarray(voxel_sh, dtype=np.float32)
    rays = np.concatenate(
        [np.asarray(rays_o, np.float32), np.asarray(rays_d, np.float32)], axis=1
    )  # [NUM_RAYS, 6]
    tvec = np.full((S + s_w, 1), 1e9, dtype=np.float32)
    tvec[:S, 0] = np.asarray(
        jnp.linspace(NEAR, FAR, S, dtype=jnp.float32), dtype=np.float32
    )
    return rays, tab, tvec


def kernel(rays_o, rays_d, voxel_density, voxel_sh):
    from concourse.bass_utils import run_bass_kernel_spmd

    n_rays = rays_o.shape[0]
    per_core = n_rays // N_CORES
    core_rays, widths = _plan(rays_o, rays_d)
    rays, tab, tvec = _host_prep(rays_o, rays_d, voxel_density, voxel_sh,
                                 max(widths))

    key = ("nc", per_core, widths)
    if key not in _CACHE:
        _CACHE[key] = build(per_core, widths)
    nc = _CACHE[key]

    in_maps = [
        {
            "rays": np.ascontiguousarray(rays[core_rays[c]]),
            "tab": tab,
            "tvec": tvec,
        }
        for c in range(N_CORES)
    ]
    res = run_bass_kernel_spmd(nc, in_maps, core_ids=list(range(N_CORES)))
    out = np.empty((n_rays, 3), np.float32)
    for c in range(N_CORES):
        out[core_rays[c]] = res.results[c]["out"]
    return out
